# revision 4
# baseline (speedup 1.0000x reference)
"""Causal self-attention (B=2, T=2048, C=2048, H=16) on 8 trn2 NeuronCores.

Sharding: core = b*4 + hg handles batch b and head-group hg (4 heads).
 - QKV projection: column-parallel over this core's 4 heads (12*128 = 1536
   output features), tokens of its batch only.
 - Attention: embarrassingly parallel over the 4 (b, h) pairs.
 - Output projection: row-parallel (this core's 512 y-channels); each core
   returns a partial [T, C] sum; the host adds the 4 partials per batch.

Host-side prep: x is transposed (to [C, T]) and cast to bf16 so the on-chip
matmuls need no transposes at all.  Scores are computed transposed
(sT[s, q] = k_chunk @ qT) so softmax probabilities come out in [s, q] layout,
which feeds both the denominator matmul (ones vector) and the AV matmul
directly.  Softmax skips the max-subtraction (logits are ~N(0, 0.8), exp is
safe in fp32) which is mathematically identical to the reference.
"""

import numpy as np
import ml_dtypes

import bass_rust
import concourse.bass as bass
import concourse.mybir as mybir
import concourse.tile as tile
from concourse.vector_clock import ScopedClock
from concourse.bass_utils import run_bass_kernel_spmd

BF = mybir.dt.bfloat16
F32 = mybir.dt.float32
AF = mybir.ActivationFunctionType
OP = mybir.AluOpType

B, T, C = 2, 2048, 2048
H, D = 16, 128
HPC = 4          # heads per core
QB = 512         # q-block (and phase-1 token chunk)
NQB = T // QB    # 4
SCALE = 1.0 / float(np.sqrt(D))
N_CORES = 8


import json as _json


def _split_sync_waits(bir: bytes, max_waits: int = 1) -> bytes:
    """This walrus build rejects instructions carrying more than one sync
    wait (Drain takes none, DMA takes few).  Move excess waits onto NoOp
    instructions inserted immediately before the carrying instruction on the
    same engine — semantically identical, the engine just stalls at the NoOp."""
    m = _json.loads(bir)
    ctr = 0
    for fn in m["functions"]:
        for blk in fn["blocks"]:
            insts = blk.get("instructions") or []
            out = []
            for inst in insts:
                si = inst.get("sync_info")
                if si:
                    waits = si.get("on_wait") or []
                    if len(waits) > max_waits:
                        extra, keep = waits[:-max_waits], waits[-max_waits:]
                        for w in extra:
                            ctr += 1
                            out.append({
                                "debug": inst.get("debug", 0),
                                "engine": inst["engine"],
                                "ins": [],
                                "name": f"I-wsplit{ctr}",
                                "opcode": "NoOp",
                                "outs": [],
                                "sync_info": {"on_update": [], "on_wait": [w]},
                            })
                        si["on_wait"] = keep
                out.append(inst)
            blk["instructions"] = out
    return _json.dumps(m).encode()


class PatchedBass(bass.Bass):
    def to_json_bytes(self, *a, **k):
        return _split_sync_waits(super().to_json_bytes(*a, **k))


class PatchedTileContext(tile.TileContext):
    """This walrus build rejects sync waits on the SP Drain (CTRL_NO_STRUCT).
    Put the end-of-kernel waits on one-wait-each NOPs ahead of the drain."""

    def _drain_and_barrier(self, tick_clock, wait_clock):
        nop0 = self.nc.sync.nop(nofuse=True)
        wait_clock.add_sem_waits(nop0.ins, ScopedClock({None: tick_clock.global_clock}))
        si = nop0.ins.sync_info
        if si is not None and len(si.on_wait) > 1:
            waits = list(si.on_wait)
            si.on_wait = waits[:1]
            for w in waits[1:]:
                n = self.nc.sync.nop(nofuse=True)
                n.ins.sync_info = bass_rust.SyncInfo(on_wait=[w], on_update=[])
        self.nc.sync.drain()
        self.nc.all_engine_barrier()
        assert self.sems is not None
        popped = self.nc._tile_sem_poison_stack.pop()
        assert popped is self._sem_poison
        self.nc.clear_and_free_semaphores(list(self.sems.allocated().values()))
        self.nc.all_engine_barrier()


def build_nc() -> bass.Bass:
    nc = PatchedBass("TRN2", target_bir_lowering=False, debug=False)

    xT_d = nc.dram_tensor("xT", [C, T], BF, kind="ExternalInput")
    wqkv_d = nc.dram_tensor("wqkv", [C, 12 * D], BF, kind="ExternalInput")
    wp_d = nc.dram_tensor("wp", [HPC * D, C], BF, kind="ExternalInput")
    tri_d = nc.dram_tensor("tri", [128, 4, QB], BF, kind="ExternalInput")
    out_d = nc.dram_tensor("out", [T, C], F32, kind="ExternalOutput")

    xT = xT_d.ap().rearrange("(co ci) t -> ci co t", ci=128)        # [128,16,T]
    wqkv = wqkv_d.ap().rearrange("(co ci) f -> ci co f", ci=128)    # [128,16,1536]
    wp = wp_d.ap().rearrange("(h di) c -> di h c", di=128)          # [128,4,C]

    with PatchedTileContext(nc) as tc:
        with tc.tile_pool(name="persist", bufs=1) as persist:
            qT_sb = persist.tile([128, HPC, T], BF, tag="qT")
            kT_sb = persist.tile([128, HPC, T], BF, tag="kT")
            v_sb = persist.tile([128, T // 128, HPC * D], BF, tag="v")
            yT_sb = persist.tile([128, HPC, T], BF, tag="yT")
            tri_sb = persist.tile([128, 4, QB], BF, tag="tri")
            ones_sb = persist.tile([128, 1], BF, tag="ones")

            nc.sync.dma_start(out=tri_sb, in_=tri_d.ap())
            nc.vector.memset(ones_sb, 1.0)

            # ---------------- Phase 1: QKV projection ----------------
            with tc.tile_pool(name="w1", bufs=1) as w1_pool, \
                 tc.tile_pool(name="xt", bufs=2) as xt_pool, \
                 tc.tile_pool(name="qkps", bufs=3, space="PSUM") as qkps, \
                 tc.tile_pool(name="vps", bufs=2, space="PSUM") as vps:
                w_sb = w1_pool.tile([128, 16, 12 * D], BF, tag="w")
                nc.sync.dma_start(out=w_sb, in_=wqkv)

                for n in range(T // QB):
                    xt = xt_pool.tile([128, 16, QB], BF, tag="xt")
                    nc.sync.dma_start(out=xt, in_=xT[:, :, n * QB:(n + 1) * QB])
                    # qT (f 0..3) and kT (f 4..7), feat-major outputs
                    for f in range(8):
                        ps = qkps.tile([128, QB], F32, tag="qkps")
                        for c in range(16):
                            nc.tensor.matmul(
                                ps,
                                w_sb[:, c, f * 128:(f + 1) * 128],
                                xt[:, c, :],
                                start=(c == 0),
                                stop=(c == 15),
                            )
                        dst = qT_sb if f < 4 else kT_sb
                        h = f % 4
                        dslice = dst[:, h, n * QB:(n + 1) * QB]
                        if f % 2 == 0:
                            nc.scalar.copy(out=dslice, in_=ps)
                        else:
                            nc.vector.tensor_copy(out=dslice, in_=ps)
                    # v in [tok, feat] layout
                    for ti in range(QB // 128):
                        ps = vps.tile([128, HPC * D], F32, tag="vps")
                        for c in range(16):
                            nc.tensor.matmul(
                                ps,
                                xt[:, c, ti * 128:(ti + 1) * 128],
                                w_sb[:, c, 8 * 128:12 * 128],
                                start=(c == 0),
                                stop=(c == 15),
                            )
                        nc.vector.tensor_copy(out=v_sb[:, n * 4 + ti, :], in_=ps)

            # ---------------- Phase 2: attention ----------------
            with tc.tile_pool(name="scps", bufs=3, space="PSUM") as scps, \
                 tc.tile_pool(name="yps", bufs=2, space="PSUM") as yps, \
                 tc.tile_pool(name="rps", bufs=2, space="PSUM") as rps, \
                 tc.tile_pool(name="pt", bufs=18, space="SBUF") as ptp, \
                 tc.tile_pool(name="rrow", bufs=4) as rrow, \
                 tc.tile_pool(name="rfull", bufs=2) as rfull:
                for h in range(HPC):
                    for m in range(NQB):
                        nch = 4 * (m + 1)
                        qsl = slice(m * QB, (m + 1) * QB)
                        # pass A: scores + exp (+ causal mask on diagonal band)
                        pts = []
                        for j in range(nch):
                            sc = scps.tile([128, QB], F32, tag="sc")
                            nc.tensor.matmul(
                                sc,
                                kT_sb[:, h, j * 128:(j + 1) * 128],
                                qT_sb[:, h, qsl],
                                start=True,
                                stop=True,
                            )
                            pt = ptp.tile([128, QB], BF, tag="pt")
                            nc.scalar.activation(out=pt, in_=sc, func=AF.Exp, scale=SCALE)
                            if j >= 4 * m:
                                nc.vector.tensor_tensor(
                                    out=pt, in0=pt, in1=tri_sb[:, j - 4 * m, :],
                                    op=OP.mult,
                                )
                            pts.append(pt)
                        # pass B: denominators and AV
                        y_ps = yps.tile([128, QB], F32, tag="y")
                        r_ps = rps.tile([1, QB], F32, tag="r")
                        for j in range(nch):
                            nc.tensor.matmul(
                                r_ps, ones_sb, pts[j],
                                start=(j == 0), stop=(j == nch - 1),
                            )
                            nc.tensor.matmul(
                                y_ps,
                                v_sb[:, j, h * D:(h + 1) * D],
                                pts[j],
                                start=(j == 0), stop=(j == nch - 1),
                            )
                        # rinv = exp(-ln(r)); broadcast to 128 partitions via DMA
                        lnr = rrow.tile([1, QB], F32, tag="lnr")
                        nc.scalar.activation(out=lnr, in_=r_ps, func=AF.Ln)
                        rinv = rrow.tile([1, QB], F32, tag="rinv")
                        nc.scalar.activation(out=rinv, in_=lnr, func=AF.Exp, scale=-1.0)
                        rf = rfull.tile([128, QB], F32, tag="rf")
                        rinv_bcast = bass.AP(
                            tensor=rinv.tensor,
                            offset=rinv.offset,
                            ap=[list(rinv.ap[0]), [0, 128]] + list(rinv.ap[-1:]),
                        )
                        nc.sync.dma_start(out=rf, in_=rinv_bcast)
                        nc.vector.tensor_tensor(
                            out=yT_sb[:, h, qsl], in0=y_ps, in1=rf, op=OP.mult,
                        )

            # ---------------- Phase 3: output projection ----------------
            with tc.tile_pool(name="wp", bufs=1) as wp_pool, \
                 tc.tile_pool(name="ops", bufs=3, space="PSUM") as ops, \
                 tc.tile_pool(name="osb", bufs=2) as osb:
                wp_sb = wp_pool.tile([128, HPC, C], BF, tag="wp")
                nc.sync.dma_start(out=wp_sb, in_=wp)
                for t in range(T // 128):
                    ot = osb.tile([128, C], F32, tag="ot")
                    for cc in range(4):
                        ps = ops.tile([128, 512], F32, tag="ops")
                        for h in range(HPC):
                            nc.tensor.matmul(
                                ps,
                                yT_sb[:, h, t * 128:(t + 1) * 128],
                                wp_sb[:, h, cc * 512:(cc + 1) * 512],
                                start=(h == 0),
                                stop=(h == HPC - 1),
                            )
                        osl = ot[:, cc * 512:(cc + 1) * 512]
                        if cc % 2 == 0:
                            nc.scalar.copy(out=osl, in_=ps)
                        else:
                            nc.vector.tensor_copy(out=osl, in_=ps)
                    nc.sync.dma_start(
                        out=out_d.ap()[t * 128:(t + 1) * 128, :], in_=ot,
                    )
    return nc


_NC = None


def _get_nc():
    global _NC
    if _NC is None:
        _NC = build_nc()
    return _NC


def make_in_maps(x, W_attn, W_proj):
    """Host-side sharding/layout prep. Returns list of 8 per-core input dicts."""
    bf = ml_dtypes.bfloat16
    x2 = np.asarray(x, dtype=np.float32)
    xT = np.ascontiguousarray(np.transpose(x2, (0, 2, 1))).astype(bf)  # [B, C, T]

    W = np.asarray(W_attn, dtype=np.float32)
    Wq, Wk, Wv = W[:, :C], W[:, C:2 * C], W[:, 2 * C:]
    Wp = np.asarray(W_proj, dtype=np.float32)

    # staircase causal masks for the 4 diagonal 128-chunks of a 512 q-block
    s_rel = np.arange(128)[:, None]
    q_rel = np.arange(QB)[None, :]
    tri = np.stack(
        [(q_rel >= s_rel + 128 * c).astype(np.float32) for c in range(4)], axis=1
    ).astype(bf)  # [128, 4, QB]

    in_maps = []
    for core in range(N_CORES):
        b, hg = core // HPC, core % HPC
        fs = slice(hg * HPC * D, (hg + 1) * HPC * D)   # this core's 512 channels
        wqkv = np.ascontiguousarray(
            np.concatenate([Wq[:, fs], Wk[:, fs], Wv[:, fs]], axis=1)
        ).astype(bf)                                    # [C, 1536]
        wp_l = np.ascontiguousarray(Wp[fs, :]).astype(bf)  # [512, C]
        in_maps.append({
            "xT": np.ascontiguousarray(xT[b]),
            "wqkv": wqkv,
            "wp": wp_l,
            "tri": np.ascontiguousarray(tri),
        })
    return in_maps


def combine_outputs(outs):
    """Sum the 4 per-head-group partials for each batch; stack to [B, T, C]."""
    out = np.empty((B, T, C), dtype=np.float32)
    for b in range(B):
        acc = outs[b * HPC].astype(np.float32, copy=True)
        for hg in range(1, HPC):
            acc += outs[b * HPC + hg]
        out[b] = acc
    return out


def kernel(x, W_attn, W_proj, mask=None):
    in_maps = make_in_maps(x, W_attn, W_proj)
    nc = _get_nc()
    res = run_bass_kernel_spmd(nc, in_maps, core_ids=list(range(N_CORES)))
    outs = [r["out"] for r in res.results]
    return combine_outputs(outs)


if __name__ == "__main__":
    rng = np.random.default_rng(0)
    x = rng.standard_normal((B, T, C), dtype=np.float32)
    W_attn = rng.standard_normal((C, 3 * C), dtype=np.float32) * 0.02
    W_proj = rng.standard_normal((C, C), dtype=np.float32) * 0.02
    out = kernel(x, W_attn, W_proj)
    print("out", out.shape, out.dtype, np.abs(out).max())
